# revision 13
# baseline (speedup 1.0000x reference)
"""Window-routed sparse attention on 8 TRN2 NeuronCores.

Sharding: 64 windows x 8 cores = 8 windows/core. Host precomputes the tiny
routing path (region means, a_r [64,64]) and the window-mixed q_m/k_m in
fp32 numpy; each core runs the heavy windowed attention relu(q_m k_m^T) v
for its 8 windows on the Tensor engine.

Structure (window pairing + PE tiling + single-consumer PSUM units):
- Windows processed in pairs: even window's c=64 channels live in SBUF
  partitions 0-63, odd window's in 64-127. A-phase matmuls (K=c=64) for the
  two windows land in PE row-groups (0,0)/(64,0) and stream CONCURRENTLY
  (row tiling, 2x). B-phase matmuls (M=c=64) write po partitions 0-63/
  64-127 -> col groups (0,0)/(0,64), also concurrent (2x).
- The relu drain (PSUM fp32 -> SBUF bf16, 2M elems/pair) is the hard
  floor: ~10us/pair across ScalarE+VectorE. Everything else hides under it.
- PSUM unit = one [128,1024] tile holding one t-half of BOTH windows
  ([w-h | x-h]), filled by 2 concurrent row-tiled mms (~390ns) and
  drained by exactly one engine (scalar: h0, vector: h1). One producer
  pair + one consumer per tile keeps the 3-tile pool ring engine-bound
  (reuse loop relu+fill+sems ~2.0us < 3 unit-cycles); per-window tiles
  (two consumers) entangle the engines through the ring (~25% slower).
- B(p) quads are emitted 2 chunks behind A(p) to fill the PE's slack;
  scalar (faster per chunk) also owns the po->SBUF output copies.
"""

import sys

sys.path.insert(0, "/opt/trn_rl_repo")

import numpy as np
import ml_dtypes

BF16 = np.dtype(ml_dtypes.bfloat16)

C = 64          # channels
NW = 64         # windows (8x8 grid of 32x32 patches on 256x256)
T = 1024        # tokens per window (32*32)
NCORES = 8
WPC = NW // NCORES   # windows per core
NPAIR = WPC // 2     # window pairs per core

_CACHE = {}

LAST_RESULT = None


def _build_program():
    import concourse.mybir as mybir
    from concourse import bacc
    from concourse.tile import TileContext

    bf16 = mybir.dt.bfloat16
    f32 = mybir.dt.float32

    nc = bacc.Bacc(None, target_bir_lowering=False)
    qm_d = nc.declare_dram_parameter("qm", [NPAIR, 128, T], bf16, isOutput=False)
    km_d = nc.declare_dram_parameter("km", [NPAIR, 128, T], bf16, isOutput=False)
    v_d = nc.declare_dram_parameter("v", [NPAIR, 128, 2, 8, C], bf16, isOutput=False)
    o_d = nc.declare_dram_parameter("o", [NPAIR, 128, T], f32, isOutput=True)

    with TileContext(nc) as tc:
        with (
            tc.tile_pool(name="qk", bufs=4) as qk_pool,
            tc.tile_pool(name="vp", bufs=2) as v_pool,
            tc.tile_pool(name="at", bufs=2) as a_pool,
            tc.tile_pool(name="ob", bufs=2) as o_pool,
            tc.tile_pool(name="pa", bufs=3, space="PSUM") as pa_pool,
            tc.tile_pool(name="po", bufs=1, space="PSUM") as po_pool,
        ):
            qk_tiles = [None] * NPAIR
            v_tiles = [None] * NPAIR
            at_tiles = [None] * NPAIR   # [128, 8, 2048] per pair
            po_tiles = [None] * NPAIR
            o_tiles = [None] * NPAIR

            def emit_dma(p):
                qm_t = qk_pool.tile([128, T], bf16, tag="qm", name=f"qm{p}")
                km_t = qk_pool.tile([128, T], bf16, tag="km", name=f"km{p}")
                v_t = v_pool.tile([128, 2, 8, C], bf16, tag="v", name=f"v{p}")
                if p == 0:
                    # cold start: first A unit only needs km cols 0:128 and
                    # qm cols 0:512 -- land those first (region-level deps).
                    # km piece rides the ACT HWDGE queue so both first
                    # pieces issue in parallel (~0.6us saved to first mm).
                    nc.scalar.dma_start(out=km_t[:, 0:128], in_=km_d[p, :, 0:128])
                    nc.sync.dma_start(out=qm_t[:, 0:512], in_=qm_d[p, :, 0:512])
                    nc.sync.dma_start(
                        out=qm_t[:, 512:1024], in_=qm_d[p, :, 512:1024]
                    )
                    nc.sync.dma_start(out=km_t[:, 128:512], in_=km_d[p, :, 128:512])
                    nc.sync.dma_start(
                        out=km_t[:, 512:1024], in_=km_d[p, :, 512:1024]
                    )
                else:
                    nc.sync.dma_start(out=qm_t, in_=qm_d[p])
                    nc.sync.dma_start(out=km_t, in_=km_d[p])
                nc.sync.dma_start(out=v_t, in_=v_d[p])
                qk_tiles[p] = (qm_t, km_t)
                v_tiles[p] = v_t

            def emit_a_chunk(p, k):
                """A-phase chunk: two h-half units, each one PSUM tile
                [w-h | x-h] filled by 2 concurrent row-tiled mms and drained
                by exactly ONE engine (scalar: h0, vector: h1). One
                producer-pair + one consumer per pool tile keeps the 3-tile
                ring's reuse loop at relu+fill (~2.0us) vs 3 unit-cycles
                (~1.9us) -- engine-bound, unlike two-consumer layouts which
                entangle the engines through the ring (~25% slower)."""
                if k == 0:
                    at_tiles[p] = a_pool.tile(
                        [128, 8, 2048], bf16, tag="attn", name=f"at{p}"
                    )
                qm_t, km_t = qk_tiles[p]
                at_t = at_tiles[p]
                kc = slice(k * 128, (k + 1) * 128)
                for h in range(2):
                    hs = slice(h * 512, (h + 1) * 512)
                    ps_h = pa_pool.tile([128, 1024], f32, tag="pa")
                    nc.tensor.matmul(
                        out=ps_h[:, 0:512],
                        lhsT=km_t[0:64, kc], rhs=qm_t[0:64, hs],
                        start=True, stop=True,
                    )
                    nc.tensor.matmul(
                        out=ps_h[:, 512:1024],
                        lhsT=km_t[64:128, kc], rhs=qm_t[64:128, hs],
                        start=True, stop=True,
                    )
                    if h == 0:
                        nc.scalar.activation(
                            out=at_t[:, k, 0:1024], in_=ps_h,
                            func=mybir.ActivationFunctionType.Relu, scale=1.0,
                        )
                    else:
                        nc.vector.tensor_scalar_max(
                            at_t[:, k, 1024:2048], ps_h, 0.0
                        )

            def emit_b_quad(p, k):
                if k == 0:
                    po_tiles[p] = po_pool.tile([128, T], f32, tag="po", name=f"po{p}")
                po = po_tiles[p]
                at_t = at_tiles[p]
                v_t = v_tiles[p]
                st, sp = (k == 0), (k == 7)
                for h in range(2):
                    hs = slice(h * 512, (h + 1) * 512)
                    a_w = slice(h * 1024, h * 1024 + 512)
                    a_x = slice(h * 1024 + 512, (h + 1) * 1024)
                    nc.tensor.matmul(
                        out=po[0:64, hs], lhsT=v_t[:, 0, k, :],
                        rhs=at_t[:, k, a_w], start=st, stop=sp,
                    )
                    nc.tensor.matmul(
                        out=po[64:128, hs], lhsT=v_t[:, 1, k, :],
                        rhs=at_t[:, k, a_x], start=st, stop=sp,
                    )

            def emit_po_copy(p, half, engine):
                if half == 0:
                    o_tiles[p] = o_pool.tile([128, T], f32, tag="o", name=f"o{p}")
                o_t = o_tiles[p]
                hs = slice(half * 512, (half + 1) * 512)
                if engine == "s":
                    nc.scalar.copy(out=o_t[:, hs], in_=po_tiles[p][:, hs])
                else:
                    nc.vector.tensor_copy(out=o_t[:, hs], in_=po_tiles[p][:, hs])
                nc.sync.dma_start(out=o_d[p, :, hs], in_=o_t[:, hs])

            emit_dma(0)
            for p in range(NPAIR):
                for k in range(8):
                    # carryover B quads lead their chunk (dep-free by now);
                    # in-pair B quads FOLLOW the chunk's A units so a B quad
                    # stalled on the po chain can't head-block unit fills.
                    if k <= 1 and p > 0:
                        emit_b_quad(p - 1, 6 + k)
                    if k == 1 and p + 1 < NPAIR:
                        emit_dma(p + 1)
                    if k == 1 and p > 0:
                        # prev pair's output drain, as early as legal (right
                        # after B(p-1,7) in program order): scalar runs them
                        # after relu(p,0), putting copy completion ahead of
                        # B(p,0)'s arrival at the PE -> no po-chain stall.
                        emit_po_copy(p - 1, 0, "s")
                        emit_po_copy(p - 1, 1, "s")
                    emit_a_chunk(p, k)
                    if k >= 2:
                        emit_b_quad(p, k - 2)
            # tail: last pair's final B quads t-half-major. The h1 quads
            # need only VECTOR relus (done early); the h0 quads wait on the
            # final scalar relu, so h1 accumulates FIRST, its halves copy +
            # DMA concurrently on both engines, and the h0 side follows in
            # quarters so the last DMA is small and issues early.
            w = NPAIR - 1
            po, at_t, v_t = po_tiles[w], at_tiles[w], v_tiles[w]
            o_t = o_pool.tile([128, T], f32, tag="o", name=f"o{w}")
            o_tiles[w] = o_t
            # h0-k6 first (needs only scalar relu(3,6), ready earliest)
            nc.tensor.matmul(
                out=po[0:64, 0:512], lhsT=v_t[:, 0, 6, :],
                rhs=at_t[:, 6, 0:512], start=False, stop=False,
            )
            nc.tensor.matmul(
                out=po[64:128, 0:512], lhsT=v_t[:, 1, 6, :],
                rhs=at_t[:, 6, 512:1024], start=False, stop=False,
            )
            for k in (6, 7):
                nc.tensor.matmul(
                    out=po[0:64, 512:1024], lhsT=v_t[:, 0, k, :],
                    rhs=at_t[:, k, 1024:1536], start=False, stop=(k == 7),
                )
                nc.tensor.matmul(
                    out=po[64:128, 512:1024], lhsT=v_t[:, 1, k, :],
                    rhs=at_t[:, k, 1536:2048], start=False, stop=(k == 7),
                )
            nc.scalar.copy(out=o_t[:, 512:768], in_=po[:, 512:768])
            nc.sync.dma_start(out=o_d[w, :, 512:768], in_=o_t[:, 512:768])
            nc.vector.tensor_copy(out=o_t[:, 768:1024], in_=po[:, 768:1024])
            nc.sync.dma_start(out=o_d[w, :, 768:1024], in_=o_t[:, 768:1024])
            nc.tensor.matmul(
                out=po[0:64, 0:512], lhsT=v_t[:, 0, 7, :],
                rhs=at_t[:, 7, 0:512], start=False, stop=True,
            )
            nc.tensor.matmul(
                out=po[64:128, 0:512], lhsT=v_t[:, 1, 7, :],
                rhs=at_t[:, 7, 512:1024], start=False, stop=True,
            )
            # final quarter drains on both engines; DMAs split across the
            # Sync and ACT HWDGE queues so their issue costs overlap.
            nc.vector.tensor_copy(out=o_t[:, 256:512], in_=po[:, 256:512])
            nc.sync.dma_start(out=o_d[w, :, 256:512], in_=o_t[:, 256:512])
            nc.scalar.copy(out=o_t[:, 0:256], in_=po[:, 0:256])
            nc.scalar.dma_start(out=o_d[w, :, 0:256], in_=o_t[:, 0:256])

    nc.finalize()
    return nc


def kernel(x, W, bias):
    import os
    from concourse.bass_utils import run_bass_kernel_spmd

    x = np.asarray(x, dtype=np.float32)
    W = np.asarray(W, dtype=np.float32)
    bias = np.asarray(bias, dtype=np.float32)

    # ---- host prep: windows, qkv, routing, mixing (tiny vs attention) ----
    xw = (
        x.reshape(C, 8, 32, 8, 32)
        .transpose(1, 3, 2, 4, 0)
        .reshape(NW, T, C)
    )
    qkv = xw @ W.T + bias  # [nw, T, 3c]
    q, k, v = qkv[..., :C], qkv[..., C:2 * C], qkv[..., 2 * C:]
    q_r = q.mean(axis=1)  # [nw, c]
    k_r = k.mean(axis=1)
    a_r = np.maximum(q_r @ k_r.T, 0.0)  # [nw, nw]
    k_m = np.tensordot(a_r, k, axes=(1, 0))  # [nw, T, c]
    q_m = np.tensordot(a_r, q, axes=(1, 0))

    if "nc" not in _CACHE:
        _CACHE["nc"] = _build_program()
    nc = _CACHE["nc"]

    NP_ALL = NW // 2  # 32 pairs across all cores
    qm_dev = np.ascontiguousarray(
        q_m.transpose(0, 2, 1).reshape(NP_ALL, 128, T)
    ).astype(BF16)
    km_dev = np.ascontiguousarray(
        k_m.transpose(0, 2, 1).reshape(NP_ALL, 128, T)
    ).astype(BF16)
    v_dev = np.ascontiguousarray(
        v.reshape(NP_ALL, 2, 8, 128, C).transpose(0, 3, 1, 2, 4)
    ).astype(BF16)

    in_maps = []
    for m in range(NCORES):
        s = slice(m * NPAIR, (m + 1) * NPAIR)
        in_maps.append({
            "qm": qm_dev[s],
            "km": km_dev[s],
            "v": v_dev[s],
        })

    trace = bool(os.environ.get("KERNEL_TRACE"))
    res = run_bass_kernel_spmd(nc, in_maps, list(range(NCORES)), trace=trace)
    global LAST_RESULT
    LAST_RESULT = res
    # unpack [npair, 2*64, T] -> [c, wpc, T]
    outs = [
        res.results[m]["o"]
        .reshape(NPAIR, 2, C, T)
        .transpose(2, 0, 1, 3)
        .reshape(C, WPC, T)
        for m in range(NCORES)
    ]
    o_cm = np.concatenate(outs, axis=1)  # [c, nw, T]

    o_img = (
        o_cm.reshape(C, 8, 8, 32, 32)
        .transpose(0, 1, 3, 2, 4)
        .reshape(1, C, 256, 256)
    )
    return o_img.astype(np.float32)
